# revision 4
# baseline (speedup 1.0000x reference)
"""Trainium2 Bass kernel for a 3-layer bidirectional GRU classifier.

Sharding: 8 cores = 4 batch shards (16 samples) x 2 directions.  Each core
runs only its own direction's recurrence (half the W_hh weight stream per
step) and computes its own direction's input projections (gi) for the next
layer.  Direction pairs exchange transposed hidden outputs per 64-step
window via pairwise AllGather so the gi GEMM can contract over the full
2H=2048 input (both directions).  Both cores of a pair compute the same
FC output for their shard; kernel() reads cores 0,2,4,6.
"""

import os
import sys

for _p in ("/opt/trn_rl_repo", "/root/.axon_site/_ro/trn_rl_repo"):
    if os.path.isdir(_p) and _p not in sys.path:
        sys.path.append(_p)

import numpy as np
import ml_dtypes

import concourse.bacc as bacc
import concourse.mybir as mybir
from concourse.tile import TileContext, add_dep_helper
from concourse.bass_utils import run_bass_kernel_spmd

BF16 = mybir.dt.bfloat16
F32 = mybir.dt.float32
AF = mybir.ActivationFunctionType
ALU = mybir.AluOpType

NCORES = 8
NSHARD = 4
H = 1024
G3 = 3 * H
CIN = 150
NCLS = 60
EPS = 1e-5
PAIRS = [[0, 1], [2, 3], [4, 5], [6, 7]]


def _perm3h(v):
    """Reorder a [..., 3072] gate-major vector to (a, gate, cj) column order."""
    a = v.reshape(*v.shape[:-1], 3, 4, 256)
    a = np.moveaxis(a, -3, -2)  # [..., 4, 3, 256]
    return a.reshape(*v.shape[:-1], G3)


def host_prep(inputs, T, n_full):
    B = n_full // NSHARD  # 16 samples per shard
    x = np.asarray(inputs["x"], np.float32)
    gamma = np.asarray(inputs["bn_gamma"], np.float32)
    beta = np.asarray(inputs["bn_beta"], np.float32)
    w_ih0 = np.asarray(inputs["w_ih0"], np.float32)
    w_hh0 = np.asarray(inputs["w_hh0"], np.float32)
    b_ih0 = np.asarray(inputs["b_ih0"], np.float32)
    b_hh0 = np.asarray(inputs["b_hh0"], np.float32)
    w_ih = np.asarray(inputs["w_ih"], np.float32)
    w_hh = np.asarray(inputs["w_hh"], np.float32)
    b_ih = np.asarray(inputs["b_ih"], np.float32)
    b_hh = np.asarray(inputs["b_hh"], np.float32)
    fc_w = np.asarray(inputs["fc_w"], np.float32)
    fc_b = np.asarray(inputs["fc_b"], np.float32)

    shared = {}
    NTF = T * n_full

    xTf = np.ascontiguousarray(x.transpose(2, 1, 0).reshape(CIN, NTF))
    shared["xtf0"] = np.ascontiguousarray(xTf[:128])
    xtf1 = np.zeros((32, NTF), np.float32)
    xtf1[: CIN - 128] = xTf[128:]
    shared["xtf1"] = xtf1

    # W_hh^T permuted bf16 per (layer, dir): [128, 8*3072], free=k*3072+a*768+g*256+cj
    whh_all = np.stack([w_hh0, w_hh[0], w_hh[1]])  # [3,2,3072,1024]
    t = whh_all.reshape(3, 2, 3, 4, 256, 8, 128)
    t = np.transpose(t, (0, 1, 6, 5, 3, 2, 4))
    whh_p = t.reshape(3, 2, 128, 8 * 3072).astype(ml_dtypes.bfloat16)

    # W_ih layers 1,2 per dir: [2, 16, 128, 3072]; k 0-7 = dir0 rows, 8-15 = dir1
    t = w_ih.reshape(2, 2, 3, 4, 256, 16, 128)
    t = np.transpose(t, (0, 5, 6, 1, 3, 2, 4))
    wih12_p = t.reshape(2, 16, 128, 2, 3072).astype(ml_dtypes.bfloat16)

    # W_ih0^T permuted fp32 [150->(128,32), 2, 3072]
    t = w_ih0.reshape(2, 3, 4, 256, CIN)
    t = np.transpose(t, (4, 0, 2, 1, 3)).reshape(CIN, 2, 3072)

    mask_rz = np.zeros(G3, np.float32)
    mask_rz[: 2 * H] = 1.0
    bias0_d = [_perm3h(b_ih0[d] + b_hh0[d] * mask_rz) for d in range(2)]
    b12_d = [[_perm3h(b_ih[li, d] + b_hh[li, d] * mask_rz) for li in range(2)]
             for d in range(2)]

    bhh_all = np.stack([b_hh0, b_hh[0], b_hh[1]])[:, :, 2 * H:]  # [3,2,1024]

    shared["fcwT"] = np.ascontiguousarray(
        fc_w.T.reshape(16, 128, NCLS).astype(ml_dtypes.bfloat16))
    shared["fcb"] = fc_b.reshape(1, NCLS).astype(ml_dtypes.bfloat16)

    shared["gamma"] = np.concatenate(
        [gamma, np.zeros(10, np.float32)]).reshape(160, 1)
    shared["beta"] = np.concatenate(
        [beta, np.zeros(10, np.float32)]).reshape(160, 1)

    shared["ones_bf"] = np.ones((1, 128), ml_dtypes.bfloat16)
    shared["ident128"] = np.eye(128, dtype=ml_dtypes.bfloat16)

    per_core = []
    for c in range(NCORES):
        s, d = c // 2, c % 2
        m = dict(shared)
        m["whh"] = np.ascontiguousarray(whh_p[:, d])  # [3, 128, 24576]
        m["wih12"] = np.ascontiguousarray(wih12_p[:, :, :, d])  # [2,16,128,3072]
        wd = np.ascontiguousarray(t[:, d])  # [150, 3072]
        m["wih0a"] = np.ascontiguousarray(wd[:128])
        w0b = np.zeros((32, 3072), np.float32)
        w0b[: CIN - 128] = wd[128:]
        m["wih0b"] = w0b
        m["bias0"] = bias0_d[d].reshape(1, 3072)
        m["bhhn"] = np.ascontiguousarray(
            bhh_all[:, d].reshape(3, 1, 1024).astype(ml_dtypes.bfloat16))
        m["bias12"] = np.stack(b12_d[d]).reshape(2, 1, 3072).astype(
            ml_dtypes.bfloat16)

        xo = x[s * B: (s + 1) * B]
        xT = xo.transpose(2, 1, 0).reshape(CIN, T * B)
        aug = np.zeros((160, T * B), np.float32)
        aug[:CIN] = xT
        aug[CIN] = 1.0
        m["xto"] = aug.astype(ml_dtypes.bfloat16)
        per_core.append(m)
    return per_core


def build_program(T, n_full):
    B = n_full // NSHARD  # 16
    ROWS = T * B
    NTF = T * n_full
    nc = bacc.Bacc("TRN2", target_bir_lowering=False, debug=False,
                   num_devices=NCORES)

    inp = {}
    def din(name, shape, dt):
        inp[name] = nc.dram_tensor(name, list(shape), dt, kind="ExternalInput")

    din("xtf0", (128, NTF), F32)
    din("xtf1", (32, NTF), F32)
    din("xto", (160, ROWS), BF16)
    din("whh", (3, 128, 8 * 3072), BF16)
    din("wih12", (2, 16, 128, 3072), BF16)
    din("wih0a", (128, 3072), F32)
    din("wih0b", (32, 3072), F32)
    din("bias0", (1, 3072), F32)
    din("bias12", (2, 1, 3072), BF16)
    din("bhhn", (3, 1, 1024), BF16)
    din("fcwT", (16, 128, NCLS), BF16)
    din("fcb", (1, NCLS), BF16)
    din("gamma", (160, 1), F32)
    din("beta", (160, 1), F32)
    din("ones_bf", (1, 128), BF16)
    din("ident128", (128, 128), BF16)

    out_t = nc.dram_tensor("out", [B, NCLS], F32, kind="ExternalOutput")

    WSTEPS = 50
    windows = []
    t0 = 0
    while t0 < T:
        windows.append((t0, min(t0 + WSTEPS, T)))
        t0 = min(t0 + WSTEPS, T)

    def mchunks(r0, r1):
        out = []
        while r0 < r1:
            out.append((r0, min(r0 + 128, r1)))
            r0 = min(r0 + 128, r1)
        return out

    with TileContext(nc) as tc:
        from contextlib import ExitStack
        ctx = ExitStack()
        pers = ctx.enter_context(tc.tile_pool(name="pers", bufs=1))
        gates_pool = ctx.enter_context(
            tc.tile_pool(name="gates", bufs=2, space="PSUM"))
        tp_pool = ctx.enter_context(
            tc.tile_pool(name="tpsum", bufs=1, space="PSUM"))
        gi_psum_pool = ctx.enter_context(
            tc.tile_pool(name="gipsum", bufs=3, space="PSUM"))
        dram_pool = ctx.enter_context(
            tc.tile_pool(name="dram", bufs=1, space="DRAM"))
        xch_pool = ctx.enter_context(
            tc.tile_pool(name="xch", bufs=2, space="DRAM"))
        gld_pool = ctx.enter_context(tc.tile_pool(name="gld", bufs=2))
        gicp_pool = ctx.enter_context(tc.tile_pool(name="gicp", bufs=2))

        ident128 = pers.tile([128, 128], BF16, tag="ident128")
        nc.sync.dma_start(ident128[:], inp["ident128"][:])
        ones_bf = pers.tile([1, 128], BF16, tag="ones")
        nc.sync.dma_start(ones_bf[:], inp["ones_bf"][:])

        gi_rz = [dram_pool.tile([T, 4, B, 512], BF16, tag=f"girz{l}",
                                name=f"gi_rz{l}") for l in range(3)]
        gi_n = [dram_pool.tile([T, 4, B, 256], BF16, tag=f"gin{l}",
                               name=f"gi_n{l}") for l in range(3)]

        def store_gi(l, g, r0, r1, cc0, cw, gsb):
            mw = r1 - r0
            if cc0 < 512:
                dstt = gi_rz[l][r0 // B: r1 // B, g][:, :, cc0: cc0 + cw]
            else:
                dstt = gi_n[l][r0 // B: r1 // B, g]
            nc.sync.dma_start(dstt, gsb[0:mw, 0:cw])

        # ---------------- phase 0: BN stats ----------------
        stats = []
        with tc.tile_pool(name="ph0s", bufs=1) as ph0s:
            for si, p in ((0, 128), (1, 32)):
                st = ph0s.tile([p, 32], F32, tag=f"st{si}", name=f"st{si}")
                stats.append(st)
                xt = ph0s.tile([p, NTF], F32, tag=f"xt{si}", name=f"xt{si}")
                nc.sync.dma_start(xt[:], inp[f"xtf{si}"][:])
                C = lambda i: st[:, i:i+1]
                nc.vector.tensor_reduce(C(0), xt[:],
                                        axis=mybir.AxisListType.X, op=ALU.add)
                nc.scalar.activation(xt[:], xt[:], AF.Square, accum_out=C(1))
                nc.vector.tensor_scalar_mul(C(2), C(0), 1.0 / NTF)
                nc.vector.tensor_scalar_mul(C(3), C(1), 1.0 / NTF)
                nc.vector.tensor_mul(C(4), C(2), C(2))
                nc.vector.tensor_sub(C(5), C(3), C(4))
                nc.vector.tensor_scalar_add(C(5), C(5), EPS)
                nc.scalar.activation(C(6), C(5), AF.Sqrt)
                nc.vector.reciprocal(C(7), C(6))
                nc.vector.tensor_mul(C(8), C(7), C(7))
                nc.vector.tensor_mul(C(9), C(5), C(8))
                nc.vector.scalar_tensor_tensor(
                    C(10), C(9), -0.5, C(7), op0=ALU.mult, op1=ALU.mult)
                nc.vector.scalar_tensor_tensor(
                    C(11), C(7), 1.5, C(10), op0=ALU.mult, op1=ALU.add)
                nc.sync.dma_start(C(12), inp["gamma"][si*128: si*128+p, :])
                nc.sync.dma_start(C(13), inp["beta"][si*128: si*128+p, :])
                nc.vector.tensor_mul(C(14), C(12), C(11))
                nc.vector.tensor_mul(C(15), C(2), C(14))
                nc.vector.tensor_sub(C(16), C(13), C(15))
            stp = [pers.tile([p_, 32], F32, tag=f"stp{si_}", name=f"stp{si_}")
                   for si_, p_ in ((0, 128), (1, 32))]
            for si in range(2):
                nc.vector.tensor_copy(stp[si][:, 0:17], stats[si][:, 0:17])

        # ---------------- phase 0b: W0 fold + gi0 (own dir) ---------
        with tc.tile_pool(name="ph0w", bufs=1) as ph0w:
            w0a = ph0w.tile([128, 3072], F32, tag="w0a", name="w0a")
            nc.sync.dma_start(w0a[:], inp["wih0a"][:])
            w0b = ph0w.tile([32, 3072], F32, tag="w0b", name="w0b")
            nc.sync.dma_start(w0b[:], inp["wih0b"][:])
            bias0 = ph0w.tile([1, 3072], F32, tag="bias0t", name="bias0t")
            nc.sync.dma_start(bias0[:], inp["bias0"][:])

            w0rows = [(w0a, 128, stp[0]), (w0b, CIN - 128, stp[1])]
            for n in range(6):
                bps = gi_psum_pool.tile([128, 512], F32, tag="gips",
                                        name=f"bps{n}")
                for ki, (w0, kp, st) in enumerate(w0rows):
                    nc.tensor.matmul(
                        bps[0:1, 0:512], st[0:kp, 16:17],
                        w0[0:kp, n * 512: (n + 1) * 512],
                        start=(ki == 0), stop=(ki == 1))
                nc.vector.tensor_add(bias0[:, n * 512: (n + 1) * 512],
                                     bps[0:1, 0:512],
                                     bias0[:, n * 512: (n + 1) * 512])
            for w0, kp, st in w0rows:
                nc.vector.tensor_scalar_mul(w0[0:kp, :], w0[0:kp, :],
                                            st[0:kp, 14:15])
            nc.sync.dma_start(w0b[CIN - 128: CIN - 128 + 1, :], bias0[:])
            w0ab = ph0w.tile([128, 3072], BF16, tag="w0ab", name="w0ab")
            w0bb = ph0w.tile([32, 3072], BF16, tag="w0bb", name="w0bb")
            nc.vector.tensor_copy(w0ab[:], w0a[:])
            nc.vector.tensor_copy(w0bb[0: CIN + 1 - 128, :],
                                  w0b[0: CIN + 1 - 128, :])

            xto_a = ph0w.tile([128, ROWS], BF16, tag="xtoa", name="xtoa")
            nc.sync.dma_start(xto_a[:], inp["xto"][0:128, :])
            xto_b = ph0w.tile([32, ROWS], BF16, tag="xtob", name="xtob")
            nc.sync.dma_start(xto_b[:], inp["xto"][128:160, :])

            for (r0, r1) in mchunks(0, ROWS):
                mw = r1 - r0
                for g in range(4):
                    for (kind, cc0, cw) in (("rz", 0, 512), ("n", 512, 256)):
                        gps = gi_psum_pool.tile(
                            [128, 512], F32, tag="gips",
                            name=f"g0ps{r0}_{g}_{kind}")
                        nc.tensor.matmul(
                            gps[0:mw, 0:cw],
                            xto_a[:, r0:r1],
                            w0ab[:, g * 768 + cc0: g * 768 + cc0 + cw],
                            start=True, stop=False)
                        nc.tensor.matmul(
                            gps[0:mw, 0:cw],
                            xto_b[0: CIN + 1 - 128, r0:r1],
                            w0bb[0: CIN + 1 - 128,
                                 g * 768 + cc0: g * 768 + cc0 + cw],
                            start=False, stop=True)
                        gsb = gicp_pool.tile(
                            [128, cw], BF16, tag=f"gisb_{kind}",
                            name=f"g0sb{r0}_{g}_{kind}")
                        nc.vector.tensor_copy(gsb[0:mw, :], gps[0:mw, 0:cw])
                        store_gi(0, g, r0, r1, cc0, cw, gsb)

        # ---------------- layers ----------------
        scan_pool = ctx.enter_context(tc.tile_pool(name="scan", bufs=1))
        owin_pool = ctx.enter_context(tc.tile_pool(name="owin", bufs=2))
        og_pool = ctx.enter_context(tc.tile_pool(name="og", bufs=2))
        outT_last = scan_pool.tile([128, 2 * 8 * B], BF16, tag="outTlast",
                                   name="outT_last")

        def exchange_window(l, ts, te, owin):
            """AllGather own outT window with pair core.

            Returns (og0, og1) gathered window tiles for l<2; fills
            outT_last for l==2 (last window only)."""
            wsz = (te - ts) * B
            tagsfx = f"{l}_{ts}"
            xin = xch_pool.tile([128, 8 * wsz], BF16, tag="xin",
                                name=f"xin_{tagsfx}")
            nc.sync.dma_start(xin[:], owin[:, 0: 8 * wsz])
            xout = xch_pool.tile([2, 128, 8 * wsz], BF16, tag="xout",
                                 name=f"xout_{tagsfx}")
            nc.gpsimd.collective_compute(
                "AllGather", ALU.bypass, replica_groups=PAIRS,
                ins=[xin[:].opt()], outs=[xout[:].opt()])
            if l == 2:
                for d in range(2):
                    nc.sync.dma_start(
                        outT_last.rearrange("p (d k r) -> p d k r",
                                            d=2, k=8)[:, d],
                        xout[d].rearrange("p (k r) -> p k r", k=8)[
                            :, :, wsz - B: wsz])
                return None
            og = [og_pool.tile([128, 8 * wsz], BF16, tag=f"ogd{d}",
                               name=f"og{d}_{tagsfx}") for d in range(2)]
            for d in range(2):
                nc.sync.dma_start(og[d][:, 0: 8 * wsz], xout[d])
            return og

        wih_pool = ctx.enter_context(tc.tile_pool(name="wih", bufs=2))

        def gi_jobs(l, ts, te, og):
            """Jobs for the layer-(l+1) input-projection GEMM of a window.

            Returns a list of closures; each emits one PE-sized chunk of
            work.  Weight-load jobs are interleaved one group ahead of the
            matmul jobs that consume them (wih_pool bufs=2 double-buffer)."""
            wsz = (te - ts) * B
            groups = [(g, kind, cc0, cw)
                      for g in range(4)
                      for (kind, cc0, cw) in (("rz", 0, 512),
                                              ("n", 512, 256))]
            tiles = {}

            def load_job(gi_idx):
                g, kind, cc0, cw = groups[gi_idx]
                col = g * 768 + cc0
                wt = wih_pool.tile([128, 16 * cw], BF16, tag="wihT",
                                   name=f"wt_{l}{ts}{g}{kind}")
                for k in range(16):
                    nc.sync.dma_start(
                        wt[:, k * cw: (k + 1) * cw],
                        inp["wih12"][l, k, :, col: col + cw])
                bt = wih_pool.tile([1, cw], BF16, tag="biasT",
                                   name=f"bt_{l}{ts}{g}{kind}")
                nc.sync.dma_start(bt[:], inp["bias12"][l, :, col: col + cw])
                tiles[gi_idx] = (wt, bt)

            def mm_job(gi_idx, r0, r1):
                g, kind, cc0, cw = groups[gi_idx]
                wt, bt = tiles[gi_idx]
                mw = r1 - r0
                q0 = r0 - ts * B
                gps = gi_psum_pool.tile(
                    [128, 512], F32, tag="gips",
                    name=f"gp_{l}{ts}{g}{kind}{r0}")
                for k in range(16):
                    dsrc, kk = k // 8, k % 8
                    nc.tensor.matmul(
                        gps[0:mw, 0:cw],
                        og[dsrc][:, kk * wsz + q0: kk * wsz + q0 + mw],
                        wt[:, k * cw: (k + 1) * cw],
                        start=(k == 0), stop=False)
                nc.tensor.matmul(
                    gps[0:mw, 0:cw], ones_bf[:, 0:mw], bt[:],
                    start=False, stop=True)

                def copy_out():
                    gsb = gicp_pool.tile(
                        [128, cw], BF16, tag="gisb",
                        name=f"gs_{l}{ts}{g}{kind}{r0}")
                    nc.vector.tensor_copy(gsb[0:mw, :], gps[0:mw, 0:cw])
                    store_gi(l + 1, g, r0, r1, cc0, cw, gsb)
                copyout_q.append(copy_out)

            jobs = []
            rcs = mchunks(ts * B, te * B)
            jobs.append(lambda: load_job(0))
            for gi_idx in range(len(groups)):
                if gi_idx + 1 < len(groups):
                    jobs.append(lambda i=gi_idx + 1: load_job(i))
                for (r0, r1) in rcs:
                    jobs.append(lambda i=gi_idx, a=r0, b=r1: mm_job(i, a, b))
            return jobs

        SCH = 8  # steps per gi-load DMA chunk
        pending = []  # gi jobs awaiting a PE gap
        copyout_q = []  # deferred gi PSUM->SBUF copy-outs
        for l in range(3):
            whh_sb = scan_pool.tile([128, 8 * 3072], BF16, tag="whh_sb",
                                    name=f"whh_sb{l}")
            for q in range(4):
                nc.sync.dma_start(whh_sb[:, q * 6144: (q + 1) * 6144],
                                  inp["whh"][l][:, q * 6144: (q + 1) * 6144])
            bhhn_sb = scan_pool.tile([1, 1024], BF16, tag="bhhn_sb",
                                     name=f"bhhn_sb{l}")
            nc.sync.dma_start(bhhn_sb[:], inp["bhhn"][l])

            zlhs = scan_pool.tile([128, B], BF16, tag="zlhs", name=f"zlhs{l}")
            nc.vector.memset(zlhs[:], 0.0)
            h_elem = [scan_pool.tile([128, 256], BF16, tag=f"h_{par}",
                                     name=f"h_{par}_{l}") for par in range(2)]
            nc.vector.memset(h_elem[0][:], 0.0)

            # scratch (bf16): rz 0:512, tb 512:768, t1 768:1024, t2 1024:1280,
            # nt 1280:1536, dd 1536:1792, zd 1792:2048
            scr = scan_pool.tile([128, 2048], BF16, tag="scr", name=f"scr{l}")

            owin_prev, wsz_prev = None, 0
            for (ts, te) in windows:
                wsz = (te - ts) * B
                nsteps = te - ts
                owin = owin_pool.tile([128, 8 * wsz], BF16, tag="owin",
                                      name=f"owin_{l}_{ts}")
                emitted = 0
                npend0 = len(pending)

                def load_chunk(t0):
                    tS = min(SCH, te - t0)
                    grz = gld_pool.tile([128, SCH * 512], BF16, tag="grz",
                                        name=f"grz_{l}_{t0}")
                    gst = gld_pool.tile([128, SCH * 256], BF16, tag="gst",
                                        name=f"gst_{l}_{t0}")
                    for g in range(4):
                        nc.sync.dma_start(
                            grz[32*g: 32*g + B, 0: tS * 512],
                            gi_rz[l][t0: t0 + tS, g].rearrange(
                                "s b c -> b s c"))
                        nc.sync.dma_start(
                            gst[32*g: 32*g + B, 0: tS * 256],
                            gi_n[l][t0: t0 + tS, g].rearrange(
                                "s b c -> b s c"))
                    return grz, gst

                def preload(t0, grz, after=None):
                    # gi_rz into the psum accumulation region (ScalarE:
                    # keeps the DVE queue free for the activation chain)
                    gp = gates_pool.tile([128, 768], F32, tag="gp",
                                         name=f"gp_{l}_{t0}")
                    so = (t0 - ts) % SCH
                    cp = nc.scalar.copy(gp[:, 0:512],
                                        grz[:, so * 512: (so + 1) * 512])
                    if after is not None:
                        # keep the copy out of the ACT queue ahead of the
                        # current step's activation chain
                        add_dep_helper(cp.ins, after.ins, sync=False,
                                       reason="preload after chain")
                    return gp

                grz, gst = load_chunk(ts)
                gp_next = preload(ts, grz)
                for t in range(ts, te):
                    so = (t - ts) % SCH
                    gp = gp_next
                    for k in range(8):
                        if t == 0:
                            lhsT = zlhs[:, 0:B]
                        elif t == ts:
                            lhsT = owin_prev[:, k * wsz_prev
                                             + wsz_prev - B:
                                             k * wsz_prev + wsz_prev]
                        else:
                            lhsT = owin[:, k * wsz + (t - 1 - ts) * B:
                                        k * wsz + (t - ts) * B]
                        for (c0, cw) in ((0, 512), (512, 256)):
                            for g in range(4):
                                nc.tensor.matmul(
                                    gp[32*g: 32*g + B, c0: c0 + cw],
                                    lhsT,
                                    whh_sb[:, k * 3072 + g * 768 + c0:
                                           k * 3072 + g * 768 + c0 + cw],
                                    start=(c0 == 512 and k == 0),
                                    stop=(c0 == 0 and k == 7),
                                    skip_group_check=True,
                                    tile_position=(0, 32 * g))
                    for g in range(4):
                        nc.tensor.matmul(
                            gp[32*g: 32*g + B, 512:768],
                            ones_bf[:, 0:B],
                            bhhn_sb[:, g * 256: (g+1) * 256],
                            start=False, stop=True,
                            skip_group_check=True,
                            tile_position=(0, 32 * g))

                    # drain pending gi jobs into the PE gap after this
                    # step's matmuls (skip early steps so the AllGather
                    # of the previous window has landed); only PE work +
                    # weight DMAs are emitted here — their DVE copy-outs
                    # flush at the end of the step
                    SKIP = 4
                    nemit = 0
                    if t - ts >= SKIP:
                        frac = (t - ts - SKIP + 1) / max(nsteps - SKIP, 1)
                        target = min(npend0, int(frac * npend0 + 0.999))
                        while emitted < target and pending:
                            pending.pop(0)()
                            emitted += 1
                            nemit += 1
                    if nemit == 0 and t - ts >= 2:
                        # queue is dry this step: issue a throwaway matmul
                        # so the PE activity monitor doesn't throttle the
                        # clock during the activation-chain gap
                        wps = gi_psum_pool.tile([128, 512], F32, tag="gips",
                                                name=f"warm_{l}_{t}")
                        for wq in range(3):
                            nc.tensor.matmul(
                                wps[0:B, 0:512], zlhs[:, 0:B],
                                whh_sb[:, wq * 512: (wq + 1) * 512],
                                start=(wq == 0), stop=(wq == 2))

                    h_prev = h_elem[t % 2]
                    h_new = h_elem[(t + 1) % 2]
                    rz = scr[:, 0:512]
                    omz = scr[:, 512:768]
                    t1, t2 = scr[:, 768:1024], scr[:, 1024:1280]
                    nt, zh = scr[:, 1280:1536], scr[:, 1536:1792]
                    u = scr[:, 1792:2048]
                    nc.scalar.activation(rz, gp[:, 0:512], AF.Sigmoid)
                    nc.vector.tensor_mul(t1, rz[:, 0:256], gp[:, 512:768])
                    nc.vector.tensor_add(t2, t1,
                                         gst[:, so * 256: (so + 1) * 256])
                    tanh_inst = nc.scalar.activation(nt, t2, AF.Tanh)
                    # (1-z) and z*h_prev computed while the tanh runs, so
                    # only two DVE ops remain after it
                    nc.vector.tensor_scalar(omz, rz[:, 256:512], -1.0, 1.0,
                                            op0=ALU.mult, op1=ALU.add)
                    nc.vector.tensor_mul(zh, rz[:, 256:512], h_prev[:])
                    nc.vector.tensor_mul(u, omz, nt)
                    nc.vector.tensor_add(h_new[:], u, zh)

                    dst = owin.rearrange(
                        "p (g j2 r) -> p g j2 r", g=4, j2=2)[
                        :, :, :, (t - ts) * B: (t - ts + 1) * B]
                    tp = tp_pool.tile([128, 256], BF16, tag="tp",
                                      name=f"tp_{l}_{t}")
                    src = tp.rearrange(
                        "p (j2 g b) -> p g j2 b", j2=2, g=4)[:, :, :, 0:B]
                    for j2 in range(2):
                        nc.tensor.transpose(
                            tp[:, j2 * 128: (j2 + 1) * 128],
                            h_new[:, j2 * 128: (j2 + 1) * 128],
                            ident128[:])
                        nc.vector.tensor_copy(dst[:, :, j2], src[:, :, j2])

                    # pipeline the next step's gi-load chunk + psum preload
                    if t + 1 < te:
                        if (t + 1 - ts) % SCH == 0:
                            grz, gst = load_chunk(t + 1)
                        gp_next = preload(t + 1, grz, after=tanh_inst)
                    while copyout_q:
                        copyout_q.pop(0)()

                # end of window: ship it; its gi jobs drain during later
                # windows' step gaps
                if l < 2 or te == T:
                    og = exchange_window(l, ts, te, owin)
                if l < 2:
                    pending.extend(gi_jobs(l, ts, te, og))
                owin_prev, wsz_prev = owin, wsz

        # flush any gi jobs not drained during step gaps
        while pending:
            pending.pop(0)()
        while copyout_q:
            copyout_q.pop(0)()

        # ---------------- FC ----------------
        fcw = pers.tile([128, 16 * NCLS], BF16, tag="fcw")
        for k in range(16):
            nc.sync.dma_start(fcw[:, k * NCLS: (k + 1) * NCLS],
                              inp["fcwT"][k])
        fcb = pers.tile([1, NCLS], BF16, tag="fcb")
        nc.sync.dma_start(fcb[:], inp["fcb"][:])
        fps = gi_psum_pool.tile([128, 512], F32, tag="gips", name="fps")
        for k in range(16):
            d, kk = k // 8, k % 8
            nc.tensor.matmul(
                fps[0:B, 0:NCLS],
                outT_last[:, (d * 8 + kk) * B: (d * 8 + kk + 1) * B],
                fcw[:, k * NCLS: (k + 1) * NCLS],
                start=(k == 0), stop=False)
        nc.tensor.matmul(fps[0:B, 0:NCLS], ones_bf[:, 0:B], fcb[:],
                         start=False, stop=True)
        fout = gicp_pool.tile([B, NCLS], F32, tag="fout")
        nc.vector.tensor_copy(fout[:], fps[0:B, 0:NCLS])
        nc.sync.dma_start(out_t[:], fout[:])

        ctx.close()

    nc.compile()
    return nc


_cache = {}


def kernel(**inputs):
    T = inputs["x"].shape[1]
    n_full = inputs["x"].shape[0]
    key = ("prog", T, n_full)
    if key not in _cache:
        _cache[key] = build_program(T, n_full)
    nc = _cache[key]
    per_core = host_prep(inputs, T, n_full)
    res = run_bass_kernel_spmd(nc, per_core, core_ids=list(range(NCORES)))
    out = np.concatenate([res.results[2 * s]["out"] for s in range(NSHARD)],
                         axis=0)
    return np.ascontiguousarray(out.astype(np.float32))
